# revision 24
# baseline (speedup 1.0000x reference)
"""Distributed MultiHeadAttention + residual + LayerNorm for 8 TRN2 NeuronCores.

Problem: B=2, S=2048, E=1024, H=16, Dh=64 (fp32 in/out).
Sharding: core c = (batch b=c//4, head-group g=c%4); each core computes 4 heads
for its batch. Output projection is row-sharded on the att dim; partials are
combined with a 4-rank ReduceScatter, then residual+LayerNorm happen on each
core's own row shard.

I/O packing: all inputs ship in three flat per-dtype blobs (fp8 / f16 / f32)
and the output is a flat f16 tensor, so each core's binding is a handful of
contiguous chunks (host-side staging cost is per-chunk + per-byte).

Device-side compute uses fp8 weights (scaled x16) with DoubleRow matmuls for
the QKV projections, bf16 attention operands, fp32 PSUM accumulation.
Scores are computed transposed (scoresT[t, s]) so softmax row sums come out of
the ctx matmul for free via a ones-column appended to V.
Mask handling: tiles are classified on the host from the actual mask input:
fully-masked tiles are skipped, each computed tile gets a column window
[c_lo, 512) excluding fully-masked columns, and a narrow band [b0, b1) where
exp() output is multiplied by a 0/1 keep matrix. The causal structure is
exploited without being hardcoded.
"""
import sys

if "/opt/trn_rl_repo" not in sys.path:
    sys.path.insert(0, "/opt/trn_rl_repo")

from contextlib import ExitStack

import numpy as np
import ml_dtypes

import concourse.bacc as bacc
import concourse.mybir as mybir
import concourse.tile as tile
from concourse.bass_utils import run_bass_kernel_spmd

BF16 = ml_dtypes.bfloat16
F8NP = ml_dtypes.float8_e4m3fn
F32 = mybir.dt.float32
F16 = mybir.dt.float16
BF = mybir.dt.bfloat16
F8 = mybir.dt.float8e4

B, S, E, H = 2, 2048, 1024, 16
Dh = E // H
LN_EPS = 1e-5
N_CORES = 8
HL = 4            # local heads per core
GROUPS = [[0, 1, 2, 3], [4, 5, 6, 7]]
NS = 4            # s blocks of 512 (query positions)
SBLK = 512
NT = 16           # t chunks of 128 (key positions)
TBLK = 128
NE = 8            # e chunks of 128 (contraction over E)
NEP = 4           # e pair-chunks of 256 (DoubleRow contraction)
W_SCALE = 16.0    # fp8 weight pre-scale (undone in PSUM->SBUF copies)
Act = mybir.ActivationFunctionType
Alu = mybir.AluOpType
USE_DR = True

# ---- flat blob layout (element offsets) --------------------------------
SZ_X = E * S
SZ_W = E * 256
O8_Q, O8_K, O8_V = 0, SZ_X, 2 * SZ_X
O8_WQ = 3 * SZ_X
O8_WK = O8_WQ + SZ_W
O8_WV = O8_WK + SZ_W
O8_WO = O8_WV + SZ_W
N8 = O8_WO + SZ_W
SZ_RES = 512 * E
NOUT = 512 * E

_BUILD_CACHE = {}


def _dn128(x):
    return (x // 128) * 128


def _up128(x):
    return -(-x // 128) * 128


def classify_mask(mask):
    """Host-side tile classification from the actual mask input.

    Returns a structure tuple:
      tiles[sb] = list of (j, c_lo, band) with band = (b0, b1, keep_idx) or None
      kw = keep tensor width
    Shared across batches (union), so one SPMD program serves all cores.
    """
    tiles = []
    mult_list = []   # (sb, j, b0, b1)
    for sb in range(NS):
        lst = []
        for j in range(NT):
            # region[b] = mask[b, s-rows, t-cols]; col c of tile = s index
            reg = mask[:, sb * SBLK:(sb + 1) * SBLK, j * TBLK:(j + 1) * TBLK]
            keep_any_col = (~reg).any(axis=2).any(axis=0)     # [SBLK] per s col
            if not keep_any_col.any():
                continue  # fully masked in every batch
            c_lo = _dn128(int(np.argmax(keep_any_col)))
            # prefix [0, c_lo) must be fully masked in all batches (guaranteed
            # since keep_any_col is False there)
            masked_any_col = reg.any(axis=2).any(axis=0)      # [SBLK]
            masked_any_col[:c_lo] = False
            if masked_any_col.any():
                nz = np.nonzero(masked_any_col)[0]
                b0 = max(c_lo, _dn128(int(nz[0])))
                b1 = min(SBLK, _up128(int(nz[-1]) + 1))
                lst.append((j, c_lo, (b0, b1, len(mult_list))))
                mult_list.append((sb, j, b0, b1))
            else:
                lst.append((j, c_lo, None))
        # widest window first so the ctx accumulation group starts with a
        # full-bank clear
        lst.sort(key=lambda t: (t[1], t[0]))
        tiles.append(tuple(lst))
    kw = max([b1 - b0 for (_, _, b0, b1) in mult_list], default=128)
    return tuple(tiles), tuple(mult_list), kw


def build(tiles, mult_list, kw, debug=False, io_only=False, collective="rs4"):
    """collective: "rs4" (per-sblock RS), "rs2" (two merged RS), "rs1" (one
    RS at the end), "none" (local DMA stand-in, timing experiments only).
    All rs* variants are numerically correct: the proj DMA writes each
    128-row chunk to the slot where the merged RS's rank-r scatter lands it,
    so rs_out keeps the same [sb*128:(sb+1)*128] row layout."""
    nm = max(1, len(mult_list))
    n16 = SZ_RES + nm * 128 * kw
    nc = bacc.Bacc("TRN2", num_devices=N_CORES)

    # ---- I/O -------------------------------------------------------------
    blob8_d = nc.dram_tensor("blob8", [N8], F8, kind="ExternalInput")
    blob16_d = nc.dram_tensor("blob16", [n16], F16, kind="ExternalInput")
    blob32_d = nc.dram_tensor("blob32", [2560], F32, kind="ExternalInput")
    out_d = nc.dram_tensor("out", [NOUT], F16, kind="ExternalOutput")

    # device views into the blobs (host pre-arranges device-friendly order)
    def xview(off):
        return blob8_d[off:off + SZ_X].rearrange(
            "(ep j p s) -> p ep j s", p=128, j=2, s=S)

    qx_v = xview(O8_Q)
    kx_v = xview(O8_K)
    vx_v = xview(O8_V)

    def wview(off):
        return blob8_d[off:off + SZ_W].rearrange(
            "(p ep j d) -> p ep j d", p=128, ep=NEP, j=2)

    wq_v, wk_v, wv_v = wview(O8_WQ), wview(O8_WK), wview(O8_WV)
    wo_v = blob8_d[O8_WO:O8_WO + SZ_W].rearrange("(p c d) -> p c d", p=128, c=2)
    res_v = blob16_d[0:SZ_RES].rearrange("(p c e) -> p c e", p=128, c=NS)
    keep_v = blob16_d[SZ_RES:SZ_RES + nm * 128 * kw].bitcast(BF).rearrange(
        "(p m s) -> p m s", p=128, m=nm)
    qb_v = blob32_d[0:256].rearrange("(p c) -> p c", c=2)
    kb_v = blob32_d[256:512].rearrange("(p c) -> p c", c=2)
    lng_v = blob32_d[512:1536].rearrange("(o e) -> o e", o=1)
    lnb_v = blob32_d[1536:2560].rearrange("(o e) -> o e", o=1)
    out_v = out_d.rearrange("(r e) -> r e", e=E)

    rs_in = nc.dram_tensor("rs_in", [S, E], BF, kind="Internal")
    rs_out = nc.dram_tensor("rs_out", [512, E], BF, kind="Internal")

    if io_only:
        with tile.TileContext(nc) as tc, ExitStack() as ctx:
            pool = ctx.enter_context(tc.tile_pool(name="io", bufs=2))
            b8 = blob8_d.rearrange("(p x) -> p x", p=128)
            t8 = pool.tile([128, N8 // 128], F8, name="t8", tag="t8")
            nc.sync.dma_start(out=t8, in_=b8)
            b16 = blob16_d.rearrange("(p x) -> p x", p=128)
            t16 = pool.tile([128, n16 // 128], F16, name="t16", tag="t16")
            nc.sync.dma_start(out=t16, in_=b16)
            b32 = blob32_d.rearrange("(p x) -> p x", p=128)
            t32 = pool.tile([128, 20], F32, name="t32", tag="t32")
            nc.sync.dma_start(out=t32, in_=b32)
            for sbx in range(4):
                oo = pool.tile([128, E], F16, name="oo", tag="oo", bufs=2)
                nc.vector.memset(oo, 0.0)
                nc.sync.dma_start(out=out_v[sbx * 128:(sbx + 1) * 128, :], in_=oo)
        nc.finalize()
        return nc

    with tile.TileContext(nc) as tc, ExitStack() as ctx:
        # ---- persistent SBUF tiles --------------------------------------
        persist = ctx.enter_context(tc.tile_pool(name="persist", bufs=1))
        q_all = persist.tile([128, 2, S], BF, name="q_all")   # [d-pair, dg, s]
        k_all = persist.tile([128, 2, S], BF, name="k_all")
        v_all = persist.tile([128, NT, HL, 65], BF, name="v_all")  # [t, j, h, d|1]
        att_sb = persist.tile([128, 2, S], BF, name="att_sb")  # [d-pair, dg, s]
        keep_sb = persist.tile([128, nm, kw], BF, name="keep_sb")
        qb_sb = persist.tile([128, 2], F32, name="qb_sb")
        kb_sb = persist.tile([128, 2], F32, name="kb_sb")
        wq_sb = persist.tile([128, NEP, 2, 256], F8, name="wq_sb")
        wk_sb = persist.tile([128, NEP, 2, 256], F8, name="wk_sb")
        wv_sb = persist.tile([128, NEP, 2, 256], F8, name="wv_sb")
        g_bc = persist.tile([128, E], F32, name="g_bc")
        b_bc = persist.tile([128, E], F32, name="b_bc")
        magic = persist.tile([128, 1], mybir.dt.uint32, name="magic")
        wo_sb = persist.tile([128, 2, E], F8, name="wo_sb")    # [d-pair, dg, e]
        resid_sb = persist.tile([128, NS, E], F16, name="resid_sb")

        nc.sync.dma_start(out=wq_sb, in_=wq_v)
        nc.sync.dma_start(out=wk_sb, in_=wk_v)
        nc.sync.dma_start(out=wv_sb, in_=wv_v)
        nc.sync.dma_start(out=qb_sb, in_=qb_v)
        nc.sync.dma_start(out=kb_sb, in_=kb_v)
        nc.gpsimd.dma_start(out=g_bc, in_=lng_v[0:1, :].to_broadcast([128, E]))
        nc.gpsimd.dma_start(out=b_bc, in_=lnb_v[0:1, :].to_broadcast([128, E]))
        nc.vector.memset(magic, 0x5F3759DF)
        # ones column for the row-sum trick
        nc.vector.memset(v_all[:, :, :, 64:65], 1.0)

        # ---- phase 1: QKV projections (fp8 DoubleRow) -------------------
        with tc.tile_pool(name="xT", bufs=3 * NEP) as xpool, \
             tc.tile_pool(name="p1", bufs=3, space="PSUM") as p1, \
             tc.tile_pool(name="p1v", bufs=3, space="PSUM") as p1v:
            def load_chunks(x_v_):
                chunks = []
                for ep in range(NEP):
                    xc = xpool.tile([128, 2, S], F8, name="xc", tag="xc")
                    nc.sync.dma_start(out=xc, in_=x_v_[:, ep, :, :])
                    chunks.append(xc)
                return chunks

            def qk_proj(chunks, w_sb_, bias_, scale_, dst_, sb):
                if True:
                    for dg in range(2):
                        ps = p1.tile([128, SBLK], F32, name="ps", tag="p1")
                        if USE_DR:
                            for s2 in range(2):
                                for ep in range(NEP):
                                    cs = slice(s2 * 256, (s2 + 1) * 256)
                                    nc.tensor.matmul(
                                        ps[:, cs],
                                        w_sb_[:, ep, :, dg * 128:(dg + 1) * 128],
                                        chunks[ep][:, :,
                                                   sb * SBLK + s2 * 256:
                                                   sb * SBLK + s2 * 256 + 256],
                                        start=(ep == 0), stop=(ep == NEP - 1),
                                        perf_mode=mybir.MatmulPerfMode.DoubleRow,
                                    )
                        else:
                            for ep in range(NEP):
                                for j in range(2):
                                    nc.tensor.matmul(
                                        ps,
                                        w_sb_[:, ep, j, dg * 128:(dg + 1) * 128],
                                        chunks[ep][:, j,
                                                   sb * SBLK:(sb + 1) * SBLK],
                                        start=(ep == 0 and j == 0),
                                        stop=(ep == NEP - 1 and j == 1),
                                    )
                        nc.scalar.activation(
                            dst_[:, dg, sb * SBLK:(sb + 1) * SBLK], ps,
                            Act.Identity, bias=bias_[:, dg:dg + 1], scale=scale_,
                        )

            nc.sync.dma_start(out=keep_sb, in_=keep_v)
            nc.sync.dma_start(out=wo_sb, in_=wo_v)
            qchunks = load_chunks(qx_v)
            kchunks = load_chunks(kx_v)
            vchunks = load_chunks(vx_v)
            nc.sync.dma_start(out=resid_sb, in_=res_v)

            def v_proj(jlist):
                # V (form A): v[t, d] = value[t, :] @ WvT
                for j in jlist:
                    ps = p1v.tile([128, 256], F32, name="psv", tag="p1v")
                    if USE_DR:
                        for ep in range(NEP):
                            nc.tensor.matmul(
                                ps,
                                vchunks[ep][:, :, j * TBLK:(j + 1) * TBLK],
                                wv_sb[:, ep, :, :],
                                start=(ep == 0), stop=(ep == NEP - 1),
                                perf_mode=mybir.MatmulPerfMode.DoubleRow,
                            )
                    else:
                        for ep in range(NEP):
                            for jj in range(2):
                                nc.tensor.matmul(
                                    ps,
                                    vchunks[ep][:, jj, j * TBLK:(j + 1) * TBLK],
                                    wv_sb[:, ep, jj, :],
                                    start=(ep == 0 and jj == 0),
                                    stop=(ep == NEP - 1 and jj == 1),
                                )
                    nc.scalar.activation(
                        v_all[:, j, :, 0:64],
                        ps.rearrange("p (h d) -> p h d", h=HL),
                        Act.Identity, scale=1.0 / W_SCALE,
                    )

            def k_proj(sb):
                for dg in range(2):
                    ps = p1.tile([128, SBLK], F32, name="ps", tag="p1")
                    if USE_DR:
                        for s2 in range(2):
                            for ep in range(NEP):
                                cs = slice(s2 * 256, (s2 + 1) * 256)
                                nc.tensor.matmul(
                                    ps[:, cs],
                                    wk_sb[:, ep, :, dg * 128:(dg + 1) * 128],
                                    kchunks[ep][:, :,
                                                sb * SBLK + s2 * 256:
                                                sb * SBLK + s2 * 256 + 256],
                                    start=(ep == 0), stop=(ep == NEP - 1),
                                    perf_mode=mybir.MatmulPerfMode.DoubleRow,
                                )
                    else:
                        for ep in range(NEP):
                            for j in range(2):
                                nc.tensor.matmul(
                                    ps,
                                    wk_sb[:, ep, j, dg * 128:(dg + 1) * 128],
                                    kchunks[ep][:, j, sb * SBLK:(sb + 1) * SBLK],
                                    start=(ep == 0 and j == 0),
                                    stop=(ep == NEP - 1 and j == 1),
                                )
                    nc.scalar.activation(
                        k_all[:, dg, sb * SBLK:(sb + 1) * SBLK], ps,
                        Act.Identity, bias=kb_sb[:, dg:dg + 1],
                        scale=1.0 / W_SCALE,
                    )

            for sb_ in range(NS):
                qk_proj(qchunks, wq_sb, qb_sb, 0.125 / W_SCALE, q_all, sb_)
            # emit in the order attention(sb) consumes: k(sb) + v(4sb..4sb+3)
            for sb_ in range(NS):
                k_proj(sb_)
                v_proj(range(4 * sb_, 4 * sb_ + 4))

        # ---- phase 2: attention; proj + RS + LN pipelined per sblock ----
        sc_pool = ctx.enter_context(tc.tile_pool(name="sc", bufs=2, space="PSUM"))
        ctx_pool = ctx.enter_context(tc.tile_pool(name="ctxp", bufs=3, space="PSUM"))
        pr_pool = ctx.enter_context(tc.tile_pool(name="prp", bufs=1, space="PSUM"))
        probs_pool = ctx.enter_context(tc.tile_pool(name="probs", bufs=6))
        small = ctx.enter_context(tc.tile_pool(name="small", bufs=4))
        work = ctx.enter_context(tc.tile_pool(name="work", bufs=4))

        def attention(sb):
            tlist = tiles[sb]
            for dg in range(2):
                cA = ctx_pool.tile([65, SBLK], F32, name="cA", tag="ctx")
                cB = ctx_pool.tile([65, SBLK], F32, name="cB", tag="ctx")
                for idx, (j, c_lo, band) in enumerate(tlist):
                    first, last = idx == 0, idx == len(tlist) - 1
                    # two-bank scores tile: head A in half 0, head B in half 1
                    sc2 = sc_pool.tile([128, 2, SBLK], F32, name="sc2", tag="sc")
                    ts = slice(j * TBLK, (j + 1) * TBLK)
                    ss = slice(sb * SBLK + c_lo, (sb + 1) * SBLK)
                    cs = slice(c_lo, SBLK)
                    nc.tensor.matmul(sc2[:, 0, cs], k_all[0:64, dg, ts],
                                     q_all[0:64, dg, ss], start=True, stop=True)
                    nc.tensor.matmul(sc2[:, 1, cs], k_all[64:128, dg, ts],
                                     q_all[64:128, dg, ss], start=True, stop=True)
                    p2 = probs_pool.tile([128, 2, SBLK], BF, name="p2", tag="pr")
                    nc.scalar.activation(p2[:, :, cs], sc2[:, :, cs], Act.Exp)
                    if band is not None:
                        b0, b1, mi = band
                        bs_ = slice(b0, b1)
                        keep_b = keep_sb[:, mi:mi + 1, 0:b1 - b0].to_broadcast(
                            [128, 2, b1 - b0])
                        nc.vector.tensor_mul(p2[:, :, bs_], p2[:, :, bs_], keep_b)
                    nc.tensor.matmul(cA[:, cs], v_all[:, j, 2 * dg, :], p2[:, 0, cs],
                                     start=first, stop=last)
                    nc.tensor.matmul(cB[:, cs], v_all[:, j, 2 * dg + 1, :],
                                     p2[:, 1, cs], start=first, stop=last)
                for i, cx in ((0, cA), (1, cB)):
                    sums = small.tile([1, SBLK], F32, name="sums", tag="sums")
                    nc.vector.tensor_copy(sums, cx[64:65, :])
                    recip = small.tile([1, SBLK], F32, name="recip", tag="recip")
                    nc.vector.reciprocal_approx_fast(recip, sums)
                    bc = work.tile([64, SBLK], F32, name="bc", tag="bc")
                    nc.gpsimd.partition_broadcast(bc, recip, channels=64)
                    nc.vector.tensor_mul(
                        att_sb[64 * i:64 * i + 64, dg, sb * SBLK:(sb + 1) * SBLK],
                        cx[0:64, :], bc)

        GROUPS_OF = {"rs4": [[0], [1], [2], [3]], "rs2": [[0, 1], [2, 3]],
                     "rs1": [[0, 1, 2, 3]], "rs31": [[0, 1, 2], [3]],
                     "none": [[0], [1], [2], [3]]}[collective]
        sb_group = {sb: grp for grp in GROUPS_OF for sb in grp}

        def proj_rs(sb):
            # out projection partials + ReduceScatter for this sblock.
            # For the last sblock attention is finished: use the freed scores
            # banks (2-bank tiles) and the idle ACT engine to shorten the tail.
            last = sb == NS - 1
            for sc in range(4):   # 128-row chunks within sblock
                srow = sb * 4 + sc
                row = slice(srow * 128, (srow + 1) * 128)
                # dest slot so that rank r of the (possibly merged) RS
                # receives rows [sb0-own, sb1-own, ...] in sb-major order,
                # matching rs_out's [sb*128:(sb+1)*128] layout.
                grp = sb_group[sb]
                dst = grp[0] * 512 + sc * (len(grp) * 128) + \
                    grp.index(sb) * 128
                pcopy = work.tile([128, E], BF, name="pcopy", tag="pcopy")
                if last:
                    pp2 = sc_pool.tile([128, 2, SBLK], F32, name="pp2", tag="sc")
                    for eo in range(2):
                        for dg in range(2):
                            nc.tensor.matmul(
                                pp2[:, eo, :],
                                att_sb[:, dg, row],
                                wo_sb[:, dg, eo * SBLK:(eo + 1) * SBLK],
                                start=(dg == 0), stop=(dg == 1),
                            )
                    nc.scalar.activation(
                        pcopy.rearrange("p (c s) -> p c s", c=2), pp2,
                        Act.Identity, scale=1.0 / W_SCALE)
                else:
                    for eo in range(2):
                        pp = pr_pool.tile([128, SBLK], F32, name="pp", tag="pp")
                        for dg in range(2):
                            nc.tensor.matmul(
                                pp,
                                att_sb[:, dg, row],
                                wo_sb[:, dg, eo * SBLK:(eo + 1) * SBLK],
                                start=(dg == 0), stop=(dg == 1),
                            )
                        nc.vector.tensor_scalar_mul(
                            pcopy[:, eo * SBLK:(eo + 1) * SBLK], pp,
                            1.0 / W_SCALE)
                nc.sync.dma_start(out=rs_in[dst:dst + 128, :], in_=pcopy)
            if collective == "none":
                nc.sync.dma_start(out=rs_out[sb * 128:(sb + 1) * 128, :],
                                  in_=rs_in[sb * SBLK:sb * SBLK + 128, :])
            elif sb == sb_group[sb][-1]:
                grp = sb_group[sb]
                lo, n = grp[0], len(grp)
                nc.gpsimd.collective_compute(
                    "ReduceScatter", Alu.add,
                    ins=[rs_in[lo * SBLK:(lo + n) * SBLK, :]],
                    outs=[rs_out[lo * 128:(lo + n) * 128, :]],
                    replica_groups=GROUPS,
                )

        def post_ln(sb):
            # residual + LN on own 128 rows of this sblock (runs one sblock
            # behind the RS so its waits never head-of-line block the queues)
            pchunk = work.tile([128, E], BF, name="pchunk", tag="pchunk")
            x_t = work.tile([128, E], F32, name="x_t", tag="x_t")
            stats = small.tile([128, 2, 6], F32, name="stats", tag="stats")
            for h in range(2):
                hs = slice(h * 512, (h + 1) * 512)
                nc.sync.dma_start(out=pchunk[:, hs],
                                  in_=rs_out[sb * 128:(sb + 1) * 128, hs])
                nc.vector.tensor_add(x_t[:, hs], resid_sb[:, sb, hs], pchunk[:, hs])
                nc.vector.bn_stats(stats[:, h, :], x_t[:, hs])
            mv = small.tile([128, 2], F32, name="mv", tag="mv")
            nc.vector.bn_aggr(mv, stats)
            # rstd = rsqrt(var + eps) on DVE (bit-trick seed + 2 Newton iters)
            # so ACT never leaves the exp table set
            U32 = mybir.dt.uint32
            ws = small.tile([128, 1], F32, name="ws", tag="ws")
            nc.vector.tensor_scalar_add(ws, mv[:, 1:2], LN_EPS)
            hbits = small.tile([128, 1], U32, name="hbits", tag="hbits")
            nc.vector.tensor_scalar(hbits, ws.bitcast(U32), 1, None,
                                    op0=Alu.logical_shift_right)
            rstd = small.tile([128, 1], F32, name="rstd", tag="rstd")
            nc.vector.scalar_tensor_tensor(
                rstd.bitcast(U32), magic, 0, hbits, op0=Alu.bypass,
                op1=Alu.subtract)
            nt = small.tile([128, 1], F32, name="nt", tag="nt")
            for _ in range(2):
                nc.vector.tensor_mul(nt, ws, rstd)
                nc.vector.tensor_mul(nt, nt, rstd)
                nc.vector.tensor_scalar(nt, nt, -0.5, 1.5, op0=Alu.mult, op1=Alu.add)
                nc.vector.tensor_mul(rstd, rstd, nt)
            y_t = work.tile([128, E], F32, name="y_t", tag="y_t")
            nc.vector.scalar_tensor_tensor(
                y_t, x_t, mv[:, 0:1], g_bc, op0=Alu.subtract, op1=Alu.mult)
            o_t = work.tile([128, E], F16, name="o_t", tag="o_t")
            nc.vector.scalar_tensor_tensor(
                o_t, y_t, rstd, b_bc, op0=Alu.mult, op1=Alu.add)
            nc.sync.dma_start(out=out_v[sb * 128:(sb + 1) * 128, :], in_=o_t)

        # Interleaved schedule: project this sblock's q/k/v, then its
        # attention (exp on ACT overlaps the next sblock's projection PE
        # work), then the PREVIOUS sblock's out-projection + RS (so the DVE
        # normalize of this sblock overlaps PE), with LN lagging the RS.
        emitted_ln = 0
        for sb in range(NS):
            attention(sb)
            proj_rs(sb)
            # LN lags one sblock behind RS readiness so its waits never
            # head-of-line block the queues
            if collective == "none":
                ready = sb + 1
            else:
                ready = 0
                for grp_ in GROUPS_OF:
                    if grp_[-1] <= sb:
                        ready = grp_[-1] + 1
            while emitted_ln < min(ready, sb):
                post_ln(emitted_ln)
                emitted_ln += 1
        for sb2 in range(emitted_ln, NS):
            post_ln(sb2)

    nc.finalize()
    return nc


def _prep_core(inputs, b, g, mult_list, kw):
    heads = slice(HL * g, HL * (g + 1))
    mask = np.asarray(inputs["mask"], bool)
    query = np.asarray(inputs["query"][b], np.float32)
    key = np.asarray(inputs["key"][b], np.float32)
    value = np.asarray(inputs["value"][b], np.float32)
    Wq_w = np.asarray(inputs["Wq_w"], np.float32)
    Wk_w = np.asarray(inputs["Wk_w"], np.float32)
    Wv_w = np.asarray(inputs["Wv_w"], np.float32)
    Wq_b = np.asarray(inputs["Wq_b"], np.float32)
    Wk_b = np.asarray(inputs["Wk_b"], np.float32)
    out_w = np.asarray(inputs["out_w"], np.float32)
    out_b = np.asarray(inputs["out_b"], np.float32)
    Wv_b = np.asarray(inputs["Wv_b"], np.float32)
    nm = max(1, len(mult_list))

    scale = np.float32(1.0 / np.sqrt(Dh))

    def packb(t):  # [4, 64] -> [128, 2] pair-major
        return np.ascontiguousarray(
            t.reshape(2, 2, Dh).transpose(1, 2, 0).reshape(128, 2))

    def wdev(W):  # [E, 256] -> [128, NEP, 2, 256] (e = ep*256 + j*128 + p)
        return np.ascontiguousarray(
            (W * W_SCALE).reshape(NEP, 2, 128, 256).transpose(2, 0, 1, 3))

    blob8 = np.empty(N8, F8NP)
    blob8[O8_Q:O8_Q + SZ_X] = np.ascontiguousarray(query.T).astype(F8NP).ravel()
    blob8[O8_K:O8_K + SZ_X] = np.ascontiguousarray(key.T).astype(F8NP).ravel()
    blob8[O8_V:O8_V + SZ_X] = np.ascontiguousarray(value.T).astype(F8NP).ravel()
    blob8[O8_WQ:O8_WQ + SZ_W] = \
        wdev(Wq_w[heads].reshape(256, E).T).astype(F8NP).ravel()
    blob8[O8_WK:O8_WK + SZ_W] = \
        wdev(Wk_w[heads].reshape(256, E).T).astype(F8NP).ravel()
    blob8[O8_WV:O8_WV + SZ_W] = \
        wdev(Wv_w[heads].reshape(256, E).T).astype(F8NP).ravel()
    # wo: [128, 2, E] with partition p = att-dim within dg half, x W_SCALE
    wo = out_w[:, 256 * g:256 * (g + 1)].T * W_SCALE   # [256, E]
    blob8[O8_WO:O8_WO + SZ_W] = np.ascontiguousarray(
        wo.reshape(2, 128, E).transpose(1, 0, 2)).astype(F8NP).ravel()

    # keep: [128, nm, kw] bf16, viewed as f16 bytes in blob16
    keep = np.zeros((nm, TBLK, kw), np.float32)
    for mi, (sb, j, b0, b1) in enumerate(mult_list):
        reg = mask[b, sb * SBLK + b0:sb * SBLK + b1, j * TBLK:(j + 1) * TBLK]
        keep[mi, :, 0:b1 - b0] = (~reg).T.astype(np.float32)
    keep_dev = np.ascontiguousarray(keep.transpose(1, 0, 2)).astype(BF16)

    const = out_b + Wv_b.reshape(E) @ out_w.T
    rows = query.reshape(NS, 4, 128, E)[:, g, :, :].reshape(512, E)
    resid = (rows + const[None, :]).reshape(NS, 128, E).transpose(1, 0, 2)

    blob16 = np.empty(SZ_RES + nm * 128 * kw, np.float16)
    blob16[0:SZ_RES] = np.ascontiguousarray(resid).astype(np.float16).ravel()
    blob16[SZ_RES:] = keep_dev.ravel().view(np.float16)

    blob32 = np.empty(2560, np.float32)
    blob32[0:256] = packb(Wq_b[heads] * scale).ravel()
    blob32[256:512] = packb(Wk_b[heads]).ravel()
    blob32[512:1536] = np.asarray(inputs["ln_g"], np.float32)
    blob32[1536:2560] = np.asarray(inputs["ln_b"], np.float32)

    return {"blob8": blob8, "blob16": blob16, "blob32": blob32}


def prep_in_maps(inputs):
    mask = np.asarray(inputs["mask"], bool)
    tiles, mult_list, kw = classify_mask(mask)
    return [_prep_core(inputs, c // 4, c % 4, mult_list, kw)
            for c in range(N_CORES)]


COLLECTIVE = "rs4"


def kernel(**inputs):
    mask = np.asarray(inputs["mask"], bool)
    tiles, mult_list, kw = classify_mask(mask)
    key_struct = (tiles, mult_list, kw, COLLECTIVE, USE_DR)
    if key_struct not in _BUILD_CACHE:
        _BUILD_CACHE[key_struct] = build(tiles, mult_list, kw,
                                         collective=COLLECTIVE)
    nc = _BUILD_CACHE[key_struct]

    in_maps = prep_in_maps(inputs)
    res = run_bass_kernel_spmd(nc, in_maps, core_ids=list(range(N_CORES)))

    out = np.empty((B, S, E), np.float32)
    for c in range(N_CORES):
        b, g = c // 4, c % 4
        o = np.asarray(res.results[c]["out"]).reshape(512, E).astype(np.float32)
        for sb in range(NS):
            out[b, sb * SBLK + 128 * g: sb * SBLK + 128 * (g + 1), :] = \
                o[sb * 128:(sb + 1) * 128, :]
    return out


# revision 25
# speedup vs baseline: 1.2173x; 1.2173x over previous
"""Distributed MultiHeadAttention + residual + LayerNorm for 8 TRN2 NeuronCores.

Problem: B=2, S=2048, E=1024, H=16, Dh=64 (fp32 in/out).
Sharding: core c = (batch b=c//4, head-group g=c%4); each core computes 4 heads
for its batch. Output projection is row-sharded on the att dim; partials are
combined with a 4-rank ReduceScatter, then residual+LayerNorm happen on each
core's own row shard.

I/O packing: all inputs ship in three flat per-dtype blobs (fp8 / f16 / f32)
and the output is a flat f16 tensor, so each core's binding is a handful of
contiguous chunks (host-side staging cost is per-chunk + per-byte).

Device-side compute uses fp8 weights (scaled x16) with DoubleRow matmuls for
the QKV projections (2x PE throughput on the 1024-deep contraction), bf16
attention operands, fp32 PSUM accumulation.  PSUM accumulation groups are
kept strictly sequential within a bank zero-region (interleaving two groups
in one 2KB region is rejected by the interpreter and corrupts results on
hardware).
Scores are computed transposed (scoresT[t, s]) so softmax row sums come out of
the ctx matmul for free via a ones-column appended to V.
Mask handling: tiles are classified on the host from the actual mask input:
fully-masked tiles are skipped, each computed tile gets a column window
[c_lo, 512) excluding fully-masked columns, and a narrow band [b0, b1) where
exp() output is multiplied by a 0/1 keep matrix. The causal structure is
exploited without being hardcoded.
"""
import sys

if "/opt/trn_rl_repo" not in sys.path:
    sys.path.insert(0, "/opt/trn_rl_repo")

from contextlib import ExitStack

import numpy as np
import ml_dtypes

import concourse.bacc as bacc
import concourse.mybir as mybir
import concourse.tile as tile
from concourse.bass_utils import run_bass_kernel_spmd

BF16 = ml_dtypes.bfloat16
F8NP = ml_dtypes.float8_e4m3fn
F32 = mybir.dt.float32
F16 = mybir.dt.float16
BF = mybir.dt.bfloat16
F8 = mybir.dt.float8e4

B, S, E, H = 2, 2048, 1024, 16
Dh = E // H
LN_EPS = 1e-5
N_CORES = 8
HL = 4            # local heads per core
GROUPS = [[0, 1, 2, 3], [4, 5, 6, 7]]
NS = 4            # s blocks of 512 (query positions)
SBLK = 512
NT = 16           # t chunks of 128 (key positions)
TBLK = 128
NE = 8            # e chunks of 128 (contraction over E)
NEP = 4           # e pair-chunks of 256 (DoubleRow contraction)
W_SCALE = 16.0    # fp8 weight pre-scale (undone in PSUM->SBUF copies)
Act = mybir.ActivationFunctionType
Alu = mybir.AluOpType
USE_DR = True

# ---- flat blob layout (element offsets) --------------------------------
SZ_X = E * S
SZ_W = E * 256
O8_Q, O8_K, O8_V = 0, SZ_X, 2 * SZ_X
O8_WQ = 3 * SZ_X
O8_WK = O8_WQ + SZ_W
O8_WV = O8_WK + SZ_W
O8_WO = O8_WV + SZ_W
N8 = O8_WO + SZ_W
SZ_RES = 512 * E
NOUT = 512 * E

_BUILD_CACHE = {}


def _dn128(x):
    return (x // 128) * 128


def _up128(x):
    return -(-x // 128) * 128


def classify_mask(mask):
    """Host-side tile classification from the actual mask input.

    Returns a structure tuple:
      tiles[sb] = list of (j, c_lo, band) with band = (b0, b1, keep_idx) or None
      kw = keep tensor width
    Shared across batches (union), so one SPMD program serves all cores.
    """
    tiles = []
    mult_list = []   # (sb, j, b0, b1)
    for sb in range(NS):
        lst = []
        for j in range(NT):
            # region[b] = mask[b, s-rows, t-cols]; col c of tile = s index
            reg = mask[:, sb * SBLK:(sb + 1) * SBLK, j * TBLK:(j + 1) * TBLK]
            keep_any_col = (~reg).any(axis=2).any(axis=0)     # [SBLK] per s col
            if not keep_any_col.any():
                continue  # fully masked in every batch
            c_lo = _dn128(int(np.argmax(keep_any_col)))
            # prefix [0, c_lo) must be fully masked in all batches (guaranteed
            # since keep_any_col is False there)
            masked_any_col = reg.any(axis=2).any(axis=0)      # [SBLK]
            masked_any_col[:c_lo] = False
            if masked_any_col.any():
                nz = np.nonzero(masked_any_col)[0]
                b0 = max(c_lo, _dn128(int(nz[0])))
                b1 = min(SBLK, _up128(int(nz[-1]) + 1))
                lst.append((j, c_lo, (b0, b1, len(mult_list))))
                mult_list.append((sb, j, b0, b1))
            else:
                lst.append((j, c_lo, None))
        # widest window first so the ctx accumulation group starts with a
        # full-bank clear
        lst.sort(key=lambda t: (t[1], t[0]))
        tiles.append(tuple(lst))
    kw = max([b1 - b0 for (_, _, b0, b1) in mult_list], default=128)
    return tuple(tiles), tuple(mult_list), kw


def build(tiles, mult_list, kw, debug=False, io_only=False, collective="rs4"):
    """collective: "rs4" (per-sblock RS), "rs2" (two merged RS), "rs1" (one
    RS at the end), "none" (local DMA stand-in, timing experiments only).
    All rs* variants are numerically correct: the proj DMA writes each
    128-row chunk to the slot where the merged RS's rank-r scatter lands it,
    so rs_out keeps the same [sb*128:(sb+1)*128] row layout."""
    nm = max(1, len(mult_list))
    n16 = SZ_RES + nm * 128 * kw
    nc = bacc.Bacc("TRN2", num_devices=N_CORES)

    # ---- I/O -------------------------------------------------------------
    blob8_d = nc.dram_tensor("blob8", [N8], F8, kind="ExternalInput")
    blob16_d = nc.dram_tensor("blob16", [n16], F16, kind="ExternalInput")
    blob32_d = nc.dram_tensor("blob32", [2560], F32, kind="ExternalInput")
    out_d = nc.dram_tensor("out", [NOUT], F16, kind="ExternalOutput")

    # device views into the blobs (host pre-arranges device-friendly order)
    def xview(off):
        return blob8_d[off:off + SZ_X].rearrange(
            "(ep j p s) -> p ep j s", p=128, j=2, s=S)

    qx_v = xview(O8_Q)
    kx_v = xview(O8_K)
    vx_v = xview(O8_V)

    def wview(off):
        return blob8_d[off:off + SZ_W].rearrange(
            "(p ep j d) -> p ep j d", p=128, ep=NEP, j=2)

    wq_v, wk_v, wv_v = wview(O8_WQ), wview(O8_WK), wview(O8_WV)
    wo_v = blob8_d[O8_WO:O8_WO + SZ_W].rearrange("(p c d) -> p c d", p=128, c=2)
    res_v = blob16_d[0:SZ_RES].rearrange("(p c e) -> p c e", p=128, c=NS)
    keep_v = blob16_d[SZ_RES:SZ_RES + nm * 128 * kw].bitcast(BF).rearrange(
        "(p m s) -> p m s", p=128, m=nm)
    qb_v = blob32_d[0:256].rearrange("(p c) -> p c", c=2)
    kb_v = blob32_d[256:512].rearrange("(p c) -> p c", c=2)
    lng_v = blob32_d[512:1536].rearrange("(o e) -> o e", o=1)
    lnb_v = blob32_d[1536:2560].rearrange("(o e) -> o e", o=1)
    out_v = out_d.rearrange("(r e) -> r e", e=E)

    rs_in = nc.dram_tensor("rs_in", [S, E], BF, kind="Internal")
    rs_out = nc.dram_tensor("rs_out", [512, E], BF, kind="Internal")

    if io_only:
        with tile.TileContext(nc) as tc, ExitStack() as ctx:
            pool = ctx.enter_context(tc.tile_pool(name="io", bufs=2))
            b8 = blob8_d.rearrange("(p x) -> p x", p=128)
            t8 = pool.tile([128, N8 // 128], F8, name="t8", tag="t8")
            nc.sync.dma_start(out=t8, in_=b8)
            b16 = blob16_d.rearrange("(p x) -> p x", p=128)
            t16 = pool.tile([128, n16 // 128], F16, name="t16", tag="t16")
            nc.sync.dma_start(out=t16, in_=b16)
            b32 = blob32_d.rearrange("(p x) -> p x", p=128)
            t32 = pool.tile([128, 20], F32, name="t32", tag="t32")
            nc.sync.dma_start(out=t32, in_=b32)
            for sbx in range(4):
                oo = pool.tile([128, E], F16, name="oo", tag="oo", bufs=2)
                nc.vector.memset(oo, 0.0)
                nc.sync.dma_start(out=out_v[sbx * 128:(sbx + 1) * 128, :], in_=oo)
        nc.finalize()
        return nc

    with tile.TileContext(nc) as tc, ExitStack() as ctx:
        # ---- persistent SBUF tiles --------------------------------------
        persist = ctx.enter_context(tc.tile_pool(name="persist", bufs=1))
        q_all = persist.tile([128, 2, S], BF, name="q_all")   # [d-pair, dg, s]
        k_all = persist.tile([128, 2, S], BF, name="k_all")
        v_all = persist.tile([128, NT, HL, 65], BF, name="v_all")  # [t, j, h, d|1]
        att_sb = persist.tile([128, 2, S], BF, name="att_sb")  # [d-pair, dg, s]
        keep_sb = persist.tile([128, nm, kw], BF, name="keep_sb")
        qb_sb = persist.tile([128, 2], F32, name="qb_sb")
        kb_sb = persist.tile([128, 2], F32, name="kb_sb")
        wq_sb = persist.tile([128, NEP, 2, 256], F8, name="wq_sb")
        wk_sb = persist.tile([128, NEP, 2, 256], F8, name="wk_sb")
        wv_sb = persist.tile([128, NEP, 2, 256], F8, name="wv_sb")
        g_bc = persist.tile([128, E], F32, name="g_bc")
        b_bc = persist.tile([128, E], F32, name="b_bc")
        magic = persist.tile([128, 1], mybir.dt.uint32, name="magic")
        wo_sb = persist.tile([128, 2, E], F8, name="wo_sb")    # [d-pair, dg, e]
        resid_sb = persist.tile([128, NS, E], F16, name="resid_sb")

        nc.sync.dma_start(out=wq_sb, in_=wq_v)
        nc.sync.dma_start(out=wk_sb, in_=wk_v)
        nc.sync.dma_start(out=wv_sb, in_=wv_v)
        nc.sync.dma_start(out=qb_sb, in_=qb_v)
        nc.sync.dma_start(out=kb_sb, in_=kb_v)
        nc.gpsimd.dma_start(out=g_bc, in_=lng_v[0:1, :].to_broadcast([128, E]))
        nc.gpsimd.dma_start(out=b_bc, in_=lnb_v[0:1, :].to_broadcast([128, E]))
        nc.vector.memset(magic, 0x5F3759DF)
        # ones column for the row-sum trick
        nc.vector.memset(v_all[:, :, :, 64:65], 1.0)

        # ---- phase 1: QKV projections (fp8 DoubleRow) -------------------
        with tc.tile_pool(name="xT", bufs=3 * NEP) as xpool, \
             tc.tile_pool(name="p1", bufs=3, space="PSUM") as p1, \
             tc.tile_pool(name="p1v", bufs=3, space="PSUM") as p1v:
            def load_chunks(x_v_):
                chunks = []
                for ep in range(NEP):
                    xc = xpool.tile([128, 2, S], F8, name="xc", tag="xc")
                    nc.sync.dma_start(out=xc, in_=x_v_[:, ep, :, :])
                    chunks.append(xc)
                return chunks

            def qk_proj(chunks, w_sb_, bias_, scale_, dst_, sb):
                if True:
                    for dg in range(2):
                        ps = p1.tile([128, SBLK], F32, name="ps", tag="p1")
                        if USE_DR:
                            for s2 in range(2):
                                for ep in range(NEP):
                                    cs = slice(s2 * 256, (s2 + 1) * 256)
                                    nc.tensor.matmul(
                                        ps[:, cs],
                                        w_sb_[:, ep, :, dg * 128:(dg + 1) * 128],
                                        chunks[ep][:, :,
                                                   sb * SBLK + s2 * 256:
                                                   sb * SBLK + s2 * 256 + 256],
                                        start=(ep == 0), stop=(ep == NEP - 1),
                                        perf_mode=mybir.MatmulPerfMode.DoubleRow,
                                    )
                        else:
                            for ep in range(NEP):
                                for j in range(2):
                                    nc.tensor.matmul(
                                        ps,
                                        w_sb_[:, ep, j, dg * 128:(dg + 1) * 128],
                                        chunks[ep][:, j,
                                                   sb * SBLK:(sb + 1) * SBLK],
                                        start=(ep == 0 and j == 0),
                                        stop=(ep == NEP - 1 and j == 1),
                                    )
                        nc.scalar.activation(
                            dst_[:, dg, sb * SBLK:(sb + 1) * SBLK], ps,
                            Act.Identity, bias=bias_[:, dg:dg + 1], scale=scale_,
                        )

            nc.sync.dma_start(out=keep_sb, in_=keep_v)
            nc.sync.dma_start(out=wo_sb, in_=wo_v)
            qchunks = load_chunks(qx_v)
            kchunks = load_chunks(kx_v)
            vchunks = load_chunks(vx_v)
            nc.sync.dma_start(out=resid_sb, in_=res_v)

            def v_proj(jlist):
                # V (form A): v[t, d] = value[t, :] @ WvT
                for j in jlist:
                    ps = p1v.tile([128, 256], F32, name="psv", tag="p1v")
                    if USE_DR:
                        for ep in range(NEP):
                            nc.tensor.matmul(
                                ps,
                                vchunks[ep][:, :, j * TBLK:(j + 1) * TBLK],
                                wv_sb[:, ep, :, :],
                                start=(ep == 0), stop=(ep == NEP - 1),
                                perf_mode=mybir.MatmulPerfMode.DoubleRow,
                            )
                    else:
                        for ep in range(NEP):
                            for jj in range(2):
                                nc.tensor.matmul(
                                    ps,
                                    vchunks[ep][:, jj, j * TBLK:(j + 1) * TBLK],
                                    wv_sb[:, ep, jj, :],
                                    start=(ep == 0 and jj == 0),
                                    stop=(ep == NEP - 1 and jj == 1),
                                )
                    nc.scalar.activation(
                        v_all[:, j, :, 0:64],
                        ps.rearrange("p (h d) -> p h d", h=HL),
                        Act.Identity, scale=1.0 / W_SCALE,
                    )

            def k_proj(sb):
                for dg in range(2):
                    ps = p1.tile([128, SBLK], F32, name="ps", tag="p1")
                    if USE_DR:
                        for s2 in range(2):
                            for ep in range(NEP):
                                cs = slice(s2 * 256, (s2 + 1) * 256)
                                nc.tensor.matmul(
                                    ps[:, cs],
                                    wk_sb[:, ep, :, dg * 128:(dg + 1) * 128],
                                    kchunks[ep][:, :,
                                                sb * SBLK + s2 * 256:
                                                sb * SBLK + s2 * 256 + 256],
                                    start=(ep == 0), stop=(ep == NEP - 1),
                                    perf_mode=mybir.MatmulPerfMode.DoubleRow,
                                )
                    else:
                        for ep in range(NEP):
                            for j in range(2):
                                nc.tensor.matmul(
                                    ps,
                                    wk_sb[:, ep, j, dg * 128:(dg + 1) * 128],
                                    kchunks[ep][:, j, sb * SBLK:(sb + 1) * SBLK],
                                    start=(ep == 0 and j == 0),
                                    stop=(ep == NEP - 1 and j == 1),
                                )
                    nc.scalar.activation(
                        k_all[:, dg, sb * SBLK:(sb + 1) * SBLK], ps,
                        Act.Identity, bias=kb_sb[:, dg:dg + 1],
                        scale=1.0 / W_SCALE,
                    )

            for sb_ in range(NS):
                qk_proj(qchunks, wq_sb, qb_sb, 0.125 / W_SCALE, q_all, sb_)
            # emit in the order attention(sb) consumes: k(sb) + v(4sb..4sb+3)
            for sb_ in range(NS):
                k_proj(sb_)
                v_proj(range(4 * sb_, 4 * sb_ + 4))

        # ---- phase 2: attention; proj + RS + LN pipelined per sblock ----
        sc_pool = ctx.enter_context(tc.tile_pool(name="sc", bufs=2, space="PSUM"))
        ctx_pool = ctx.enter_context(tc.tile_pool(name="ctxp", bufs=3, space="PSUM"))
        pr_pool = ctx.enter_context(tc.tile_pool(name="prp", bufs=1, space="PSUM"))
        probs_pool = ctx.enter_context(tc.tile_pool(name="probs", bufs=6))
        small = ctx.enter_context(tc.tile_pool(name="small", bufs=4))
        work = ctx.enter_context(tc.tile_pool(name="work", bufs=4))

        def attention(sb):
            tlist = tiles[sb]
            for dg in range(2):
                cA = ctx_pool.tile([65, SBLK], F32, name="cA", tag="ctx")
                cB = ctx_pool.tile([65, SBLK], F32, name="cB", tag="ctx")
                for idx, (j, c_lo, band) in enumerate(tlist):
                    first, last = idx == 0, idx == len(tlist) - 1
                    # two-bank scores tile: head A in half 0, head B in half 1
                    sc2 = sc_pool.tile([128, 2, SBLK], F32, name="sc2", tag="sc")
                    ts = slice(j * TBLK, (j + 1) * TBLK)
                    ss = slice(sb * SBLK + c_lo, (sb + 1) * SBLK)
                    cs = slice(c_lo, SBLK)
                    nc.tensor.matmul(sc2[:, 0, cs], k_all[0:64, dg, ts],
                                     q_all[0:64, dg, ss], start=True, stop=True)
                    nc.tensor.matmul(sc2[:, 1, cs], k_all[64:128, dg, ts],
                                     q_all[64:128, dg, ss], start=True, stop=True)
                    p2 = probs_pool.tile([128, 2, SBLK], BF, name="p2", tag="pr")
                    nc.scalar.activation(p2[:, :, cs], sc2[:, :, cs], Act.Exp)
                    if band is not None:
                        b0, b1, mi = band
                        bs_ = slice(b0, b1)
                        keep_b = keep_sb[:, mi:mi + 1, 0:b1 - b0].to_broadcast(
                            [128, 2, b1 - b0])
                        nc.vector.tensor_mul(p2[:, :, bs_], p2[:, :, bs_], keep_b)
                    nc.tensor.matmul(cA[:, cs], v_all[:, j, 2 * dg, :], p2[:, 0, cs],
                                     start=first, stop=last)
                    nc.tensor.matmul(cB[:, cs], v_all[:, j, 2 * dg + 1, :],
                                     p2[:, 1, cs], start=first, stop=last)
                for i, cx in ((0, cA), (1, cB)):
                    sums = small.tile([1, SBLK], F32, name="sums", tag="sums")
                    nc.vector.tensor_copy(sums, cx[64:65, :])
                    recip = small.tile([1, SBLK], F32, name="recip", tag="recip")
                    nc.vector.reciprocal_approx_fast(recip, sums)
                    bc = work.tile([64, SBLK], F32, name="bc", tag="bc")
                    nc.gpsimd.partition_broadcast(bc, recip, channels=64)
                    nc.vector.tensor_mul(
                        att_sb[64 * i:64 * i + 64, dg, sb * SBLK:(sb + 1) * SBLK],
                        cx[0:64, :], bc)

        GROUPS_OF = {"rs4": [[0], [1], [2], [3]], "rs2": [[0, 1], [2, 3]],
                     "rs1": [[0, 1, 2, 3]], "rs31": [[0, 1, 2], [3]],
                     "none": [[0], [1], [2], [3]]}[collective]
        sb_group = {sb: grp for grp in GROUPS_OF for sb in grp}

        def proj_rs(sb):
            # out projection partials + ReduceScatter for this sblock.
            # For the last sblock attention is finished: use the freed scores
            # banks (2-bank tiles) and the idle ACT engine to shorten the tail.
            last = sb == NS - 1
            for sc in range(4):   # 128-row chunks within sblock
                srow = sb * 4 + sc
                row = slice(srow * 128, (srow + 1) * 128)
                # dest slot so that rank r of the (possibly merged) RS
                # receives rows [sb0-own, sb1-own, ...] in sb-major order,
                # matching rs_out's [sb*128:(sb+1)*128] layout.
                grp = sb_group[sb]
                dst = grp[0] * 512 + sc * (len(grp) * 128) + \
                    grp.index(sb) * 128
                pcopy = work.tile([128, E], BF, name="pcopy", tag="pcopy")
                if last:
                    pp2 = sc_pool.tile([128, 2, SBLK], F32, name="pp2", tag="sc")
                    for eo in range(2):
                        for dg in range(2):
                            nc.tensor.matmul(
                                pp2[:, eo, :],
                                att_sb[:, dg, row],
                                wo_sb[:, dg, eo * SBLK:(eo + 1) * SBLK],
                                start=(dg == 0), stop=(dg == 1),
                            )
                    nc.scalar.activation(
                        pcopy.rearrange("p (c s) -> p c s", c=2), pp2,
                        Act.Identity, scale=1.0 / W_SCALE)
                else:
                    for eo in range(2):
                        pp = pr_pool.tile([128, SBLK], F32, name="pp", tag="pp")
                        for dg in range(2):
                            nc.tensor.matmul(
                                pp,
                                att_sb[:, dg, row],
                                wo_sb[:, dg, eo * SBLK:(eo + 1) * SBLK],
                                start=(dg == 0), stop=(dg == 1),
                            )
                        nc.vector.tensor_scalar_mul(
                            pcopy[:, eo * SBLK:(eo + 1) * SBLK], pp,
                            1.0 / W_SCALE)
                nc.sync.dma_start(out=rs_in[dst:dst + 128, :], in_=pcopy)
            if collective == "none":
                nc.sync.dma_start(out=rs_out[sb * 128:(sb + 1) * 128, :],
                                  in_=rs_in[sb * SBLK:sb * SBLK + 128, :])
            elif sb == sb_group[sb][-1]:
                grp = sb_group[sb]
                lo, n = grp[0], len(grp)
                nc.gpsimd.collective_compute(
                    "ReduceScatter", Alu.add,
                    ins=[rs_in[lo * SBLK:(lo + n) * SBLK, :]],
                    outs=[rs_out[lo * 128:(lo + n) * 128, :]],
                    replica_groups=GROUPS,
                )

        def post_ln(sb):
            # residual + LN on own 128 rows of this sblock (runs one sblock
            # behind the RS so its waits never head-of-line block the queues)
            pchunk = work.tile([128, E], BF, name="pchunk", tag="pchunk")
            x_t = work.tile([128, E], F32, name="x_t", tag="x_t")
            stats = small.tile([128, 2, 6], F32, name="stats", tag="stats")
            for h in range(2):
                hs = slice(h * 512, (h + 1) * 512)
                nc.sync.dma_start(out=pchunk[:, hs],
                                  in_=rs_out[sb * 128:(sb + 1) * 128, hs])
                nc.vector.tensor_add(x_t[:, hs], resid_sb[:, sb, hs], pchunk[:, hs])
                nc.vector.bn_stats(stats[:, h, :], x_t[:, hs])
            mv = small.tile([128, 2], F32, name="mv", tag="mv")
            nc.vector.bn_aggr(mv, stats)
            # rstd = rsqrt(var + eps) on DVE (bit-trick seed + 2 Newton iters)
            # so ACT never leaves the exp table set
            U32 = mybir.dt.uint32
            ws = small.tile([128, 1], F32, name="ws", tag="ws")
            nc.vector.tensor_scalar_add(ws, mv[:, 1:2], LN_EPS)
            hbits = small.tile([128, 1], U32, name="hbits", tag="hbits")
            nc.vector.tensor_scalar(hbits, ws.bitcast(U32), 1, None,
                                    op0=Alu.logical_shift_right)
            rstd = small.tile([128, 1], F32, name="rstd", tag="rstd")
            nc.vector.scalar_tensor_tensor(
                rstd.bitcast(U32), magic, 0, hbits, op0=Alu.bypass,
                op1=Alu.subtract)
            nt = small.tile([128, 1], F32, name="nt", tag="nt")
            for _ in range(2):
                nc.vector.tensor_mul(nt, ws, rstd)
                nc.vector.tensor_mul(nt, nt, rstd)
                nc.vector.tensor_scalar(nt, nt, -0.5, 1.5, op0=Alu.mult, op1=Alu.add)
                nc.vector.tensor_mul(rstd, rstd, nt)
            y_t = work.tile([128, E], F32, name="y_t", tag="y_t")
            nc.vector.scalar_tensor_tensor(
                y_t, x_t, mv[:, 0:1], g_bc, op0=Alu.subtract, op1=Alu.mult)
            o_t = work.tile([128, E], F16, name="o_t", tag="o_t")
            nc.vector.scalar_tensor_tensor(
                o_t, y_t, rstd, b_bc, op0=Alu.mult, op1=Alu.add)
            nc.sync.dma_start(out=out_v[sb * 128:(sb + 1) * 128, :], in_=o_t)

        # Interleaved schedule: project this sblock's q/k/v, then its
        # attention (exp on ACT overlaps the next sblock's projection PE
        # work), then the PREVIOUS sblock's out-projection + RS (so the DVE
        # normalize of this sblock overlaps PE), with LN lagging the RS.
        emitted_ln = 0
        for sb in range(NS):
            attention(sb)
            proj_rs(sb)
            # LN lags one sblock behind RS readiness so its waits never
            # head-of-line block the queues
            if collective == "none":
                ready = sb + 1
            else:
                ready = 0
                for grp_ in GROUPS_OF:
                    if grp_[-1] <= sb:
                        ready = grp_[-1] + 1
            while emitted_ln < min(ready, sb):
                post_ln(emitted_ln)
                emitted_ln += 1
        for sb2 in range(emitted_ln, NS):
            post_ln(sb2)

    nc.finalize()
    return nc


def _prep_core(inputs, b, g, mult_list, kw):
    heads = slice(HL * g, HL * (g + 1))
    mask = np.asarray(inputs["mask"], bool)
    query = np.asarray(inputs["query"][b], np.float32)
    key = np.asarray(inputs["key"][b], np.float32)
    value = np.asarray(inputs["value"][b], np.float32)
    Wq_w = np.asarray(inputs["Wq_w"], np.float32)
    Wk_w = np.asarray(inputs["Wk_w"], np.float32)
    Wv_w = np.asarray(inputs["Wv_w"], np.float32)
    Wq_b = np.asarray(inputs["Wq_b"], np.float32)
    Wk_b = np.asarray(inputs["Wk_b"], np.float32)
    out_w = np.asarray(inputs["out_w"], np.float32)
    out_b = np.asarray(inputs["out_b"], np.float32)
    Wv_b = np.asarray(inputs["Wv_b"], np.float32)
    nm = max(1, len(mult_list))

    scale = np.float32(1.0 / np.sqrt(Dh))

    def packb(t):  # [4, 64] -> [128, 2] pair-major
        return np.ascontiguousarray(
            t.reshape(2, 2, Dh).transpose(1, 2, 0).reshape(128, 2))

    def wdev(W):  # [E, 256] -> [128, NEP, 2, 256] (e = ep*256 + j*128 + p)
        return np.ascontiguousarray(
            (W * W_SCALE).reshape(NEP, 2, 128, 256).transpose(2, 0, 1, 3))

    blob8 = np.empty(N8, F8NP)
    blob8[O8_Q:O8_Q + SZ_X] = np.ascontiguousarray(query.T).astype(F8NP).ravel()
    blob8[O8_K:O8_K + SZ_X] = np.ascontiguousarray(key.T).astype(F8NP).ravel()
    blob8[O8_V:O8_V + SZ_X] = np.ascontiguousarray(value.T).astype(F8NP).ravel()
    blob8[O8_WQ:O8_WQ + SZ_W] = \
        wdev(Wq_w[heads].reshape(256, E).T).astype(F8NP).ravel()
    blob8[O8_WK:O8_WK + SZ_W] = \
        wdev(Wk_w[heads].reshape(256, E).T).astype(F8NP).ravel()
    blob8[O8_WV:O8_WV + SZ_W] = \
        wdev(Wv_w[heads].reshape(256, E).T).astype(F8NP).ravel()
    # wo: [128, 2, E] with partition p = att-dim within dg half, x W_SCALE
    wo = out_w[:, 256 * g:256 * (g + 1)].T * W_SCALE   # [256, E]
    blob8[O8_WO:O8_WO + SZ_W] = np.ascontiguousarray(
        wo.reshape(2, 128, E).transpose(1, 0, 2)).astype(F8NP).ravel()

    # keep: [128, nm, kw] bf16, viewed as f16 bytes in blob16
    keep = np.zeros((nm, TBLK, kw), np.float32)
    for mi, (sb, j, b0, b1) in enumerate(mult_list):
        reg = mask[b, sb * SBLK + b0:sb * SBLK + b1, j * TBLK:(j + 1) * TBLK]
        keep[mi, :, 0:b1 - b0] = (~reg).T.astype(np.float32)
    keep_dev = np.ascontiguousarray(keep.transpose(1, 0, 2)).astype(BF16)

    const = out_b + Wv_b.reshape(E) @ out_w.T
    rows = query.reshape(NS, 4, 128, E)[:, g, :, :].reshape(512, E)
    resid = (rows + const[None, :]).reshape(NS, 128, E).transpose(1, 0, 2)

    blob16 = np.empty(SZ_RES + nm * 128 * kw, np.float16)
    blob16[0:SZ_RES] = np.ascontiguousarray(resid).astype(np.float16).ravel()
    blob16[SZ_RES:] = keep_dev.ravel().view(np.float16)

    blob32 = np.empty(2560, np.float32)
    blob32[0:256] = packb(Wq_b[heads] * scale).ravel()
    blob32[256:512] = packb(Wk_b[heads]).ravel()
    blob32[512:1536] = np.asarray(inputs["ln_g"], np.float32)
    blob32[1536:2560] = np.asarray(inputs["ln_b"], np.float32)

    return {"blob8": blob8, "blob16": blob16, "blob32": blob32}


def prep_in_maps(inputs):
    mask = np.asarray(inputs["mask"], bool)
    tiles, mult_list, kw = classify_mask(mask)
    return [_prep_core(inputs, c // 4, c % 4, mult_list, kw)
            for c in range(N_CORES)]


COLLECTIVE = "rs4"


def kernel(**inputs):
    mask = np.asarray(inputs["mask"], bool)
    tiles, mult_list, kw = classify_mask(mask)
    key_struct = (tiles, mult_list, kw, COLLECTIVE, USE_DR)
    if key_struct not in _BUILD_CACHE:
        _BUILD_CACHE[key_struct] = build(tiles, mult_list, kw,
                                         collective=COLLECTIVE)
    nc = _BUILD_CACHE[key_struct]

    in_maps = prep_in_maps(inputs)
    res = run_bass_kernel_spmd(nc, in_maps, core_ids=list(range(N_CORES)))

    out = np.empty((B, S, E), np.float32)
    for c in range(N_CORES):
        b, g = c // 4, c % 4
        o = np.asarray(res.results[c]["out"]).reshape(512, E).astype(np.float32)
        for sb in range(NS):
            out[b, sb * SBLK + 128 * g: sb * SBLK + 128 * (g + 1), :] = \
                o[sb * 128:(sb + 1) * 128, :]
    return out


# revision 28
# speedup vs baseline: 1.7310x; 1.4220x over previous
"""Distributed MultiHeadAttention + residual + LayerNorm for 8 TRN2 NeuronCores.

Problem: B=2, S=2048, E=1024, H=16, Dh=64 (fp32 in/out).
Sharding: core c = (batch b=c//4, head-group g=c%4); each core computes 4 heads
for its batch. Output projection is row-sharded on the att dim; partials are
combined with a 4-rank ReduceScatter, then residual+LayerNorm happen on each
core's own row shard.

I/O packing: all inputs ship in three flat per-dtype blobs (fp8 / f16 / f32)
and the output is a flat f16 tensor, so each core's binding is a handful of
contiguous chunks (host-side staging cost is per-chunk + per-byte).

Device-side compute uses fp8 weights (scaled x16) with DoubleRow matmuls for
the QKV projections (2x PE throughput on the 1024-deep contraction), bf16
attention operands, fp32 PSUM accumulation.  PSUM accumulation groups are
kept strictly sequential within a bank zero-region (interleaving two groups
in one 2KB region is rejected by the interpreter and corrupts results on
hardware).
Scores are computed transposed (scoresT[t, s]) so softmax row sums come out of
the ctx matmul for free via a ones-column appended to V.
Mask handling: tiles are classified on the host from the actual mask input:
fully-masked tiles are skipped, each computed tile gets a column window
[c_lo, 512) excluding fully-masked columns, and a narrow band [b0, b1) where
exp() output is multiplied by a 0/1 keep matrix. The causal structure is
exploited without being hardcoded.
"""
import sys

if "/opt/trn_rl_repo" not in sys.path:
    sys.path.insert(0, "/opt/trn_rl_repo")

from contextlib import ExitStack

import numpy as np
import ml_dtypes

import concourse.bacc as bacc
import concourse.mybir as mybir
import concourse.tile as tile
from concourse.bass_utils import run_bass_kernel_spmd

BF16 = ml_dtypes.bfloat16
F8NP = ml_dtypes.float8_e4m3fn
F32 = mybir.dt.float32
F16 = mybir.dt.float16
BF = mybir.dt.bfloat16
F8 = mybir.dt.float8e4

B, S, E, H = 2, 2048, 1024, 16
Dh = E // H
LN_EPS = 1e-5
N_CORES = 8
HL = 4            # local heads per core
GROUPS = [[0, 1, 2, 3], [4, 5, 6, 7]]
NS = 4            # s blocks of 512 (query positions)
SBLK = 512
NT = 16           # t chunks of 128 (key positions)
TBLK = 128
NE = 8            # e chunks of 128 (contraction over E)
NEP = 4           # e pair-chunks of 256 (DoubleRow contraction)
W_SCALE = 16.0    # fp8 weight pre-scale (undone in PSUM->SBUF copies)
Act = mybir.ActivationFunctionType
Alu = mybir.AluOpType
USE_DR = True

# ---- flat blob layout (element offsets) --------------------------------
SZ_X = E * S
SZ_W = E * 256
O8_Q, O8_K, O8_V = 0, SZ_X, 2 * SZ_X
O8_WQ = 3 * SZ_X
O8_WK = O8_WQ + SZ_W
O8_WV = O8_WK + SZ_W
O8_WO = O8_WV + SZ_W
N8 = O8_WO + SZ_W
SZ_RES = 512 * E
NOUT = 512 * E

_BUILD_CACHE = {}


def _dn128(x):
    return (x // 128) * 128


def _up128(x):
    return -(-x // 128) * 128


def classify_mask(mask):
    """Host-side tile classification from the actual mask input.

    Returns a structure tuple:
      tiles[sb] = list of (j, c_lo, band) with band = (b0, b1, keep_idx) or None
      kw = keep tensor width
    Shared across batches (union), so one SPMD program serves all cores.
    """
    tiles = []
    mult_list = []   # (sb, j, b0, b1)
    for sb in range(NS):
        lst = []
        for j in range(NT):
            # region[b] = mask[b, s-rows, t-cols]; col c of tile = s index
            reg = mask[:, sb * SBLK:(sb + 1) * SBLK, j * TBLK:(j + 1) * TBLK]
            keep_any_col = (~reg).any(axis=2).any(axis=0)     # [SBLK] per s col
            if not keep_any_col.any():
                continue  # fully masked in every batch
            c_lo = _dn128(int(np.argmax(keep_any_col)))
            # prefix [0, c_lo) must be fully masked in all batches (guaranteed
            # since keep_any_col is False there)
            masked_any_col = reg.any(axis=2).any(axis=0)      # [SBLK]
            masked_any_col[:c_lo] = False
            if masked_any_col.any():
                nz = np.nonzero(masked_any_col)[0]
                b0 = max(c_lo, _dn128(int(nz[0])))
                b1 = min(SBLK, _up128(int(nz[-1]) + 1))
                lst.append((j, c_lo, (b0, b1, len(mult_list))))
                mult_list.append((sb, j, b0, b1))
            else:
                lst.append((j, c_lo, None))
        # widest window first so the ctx accumulation group starts with a
        # full-bank clear
        lst.sort(key=lambda t: (t[1], t[0]))
        tiles.append(tuple(lst))
    kw = max([b1 - b0 for (_, _, b0, b1) in mult_list], default=128)
    return tuple(tiles), tuple(mult_list), kw


def build(tiles, mult_list, kw, debug=False, io_only=False, collective="rs4"):
    """collective: "rs4" (per-sblock RS), "rs2" (two merged RS), "rs1" (one
    RS at the end), "none" (local DMA stand-in, timing experiments only).
    All rs* variants are numerically correct: the proj DMA writes each
    128-row chunk to the slot where the merged RS's rank-r scatter lands it,
    so rs_out keeps the same [sb*128:(sb+1)*128] row layout."""
    nm = max(1, len(mult_list))
    n16 = SZ_RES + nm * 128 * kw
    nc = bacc.Bacc("TRN2", num_devices=N_CORES)

    # ---- I/O -------------------------------------------------------------
    blob8_d = nc.dram_tensor("blob8", [N8], F8, kind="ExternalInput")
    blob16_d = nc.dram_tensor("blob16", [n16], F16, kind="ExternalInput")
    blob32_d = nc.dram_tensor("blob32", [2560], F32, kind="ExternalInput")
    out_d = nc.dram_tensor("out", [NOUT], F16, kind="ExternalOutput")

    # device views into the blobs (host pre-arranges device-friendly order)
    def xview(off):
        return blob8_d[off:off + SZ_X].rearrange(
            "(ep j p s) -> p ep j s", p=128, j=2, s=S)

    qx_v = xview(O8_Q)
    kx_v = xview(O8_K)
    vx_v = xview(O8_V)

    def wview(off):
        return blob8_d[off:off + SZ_W].rearrange(
            "(p ep j d) -> p ep j d", p=128, ep=NEP, j=2)

    wq_v, wk_v, wv_v = wview(O8_WQ), wview(O8_WK), wview(O8_WV)
    wo_v = blob8_d[O8_WO:O8_WO + SZ_W].rearrange("(p c d) -> p c d", p=128, c=2)
    res_v = blob16_d[0:SZ_RES].rearrange("(p c e) -> p c e", p=128, c=NS)
    keep_v = blob16_d[SZ_RES:SZ_RES + nm * 128 * kw].bitcast(BF).rearrange(
        "(p m s) -> p m s", p=128, m=nm)
    qb_v = blob32_d[0:256].rearrange("(p c) -> p c", c=2)
    kb_v = blob32_d[256:512].rearrange("(p c) -> p c", c=2)
    lng_v = blob32_d[512:1536].rearrange("(o e) -> o e", o=1)
    lnb_v = blob32_d[1536:2560].rearrange("(o e) -> o e", o=1)
    out_v = out_d.rearrange("(r e) -> r e", e=E)

    rs_in = nc.dram_tensor("rs_in", [S, E], BF, kind="Internal")
    rs_out = nc.dram_tensor("rs_out", [512, E], BF, kind="Internal")

    if io_only:
        with tile.TileContext(nc) as tc, ExitStack() as ctx:
            pool = ctx.enter_context(tc.tile_pool(name="io", bufs=2))
            b8 = blob8_d.rearrange("(p x) -> p x", p=128)
            t8 = pool.tile([128, N8 // 128], F8, name="t8", tag="t8")
            nc.sync.dma_start(out=t8, in_=b8)
            b16 = blob16_d.rearrange("(p x) -> p x", p=128)
            t16 = pool.tile([128, n16 // 128], F16, name="t16", tag="t16")
            nc.sync.dma_start(out=t16, in_=b16)
            b32 = blob32_d.rearrange("(p x) -> p x", p=128)
            t32 = pool.tile([128, 20], F32, name="t32", tag="t32")
            nc.sync.dma_start(out=t32, in_=b32)
            for sbx in range(4):
                oo = pool.tile([128, E], F16, name="oo", tag="oo", bufs=2)
                nc.vector.memset(oo, 0.0)
                nc.sync.dma_start(out=out_v[sbx * 128:(sbx + 1) * 128, :], in_=oo)
        nc.finalize()
        return nc

    with tile.TileContext(nc) as tc, ExitStack() as ctx:
        # ---- persistent SBUF tiles --------------------------------------
        persist = ctx.enter_context(tc.tile_pool(name="persist", bufs=1))
        q_all = persist.tile([128, 2, S], BF, name="q_all")   # [d-pair, dg, s]
        k_all = persist.tile([128, 2, S], BF, name="k_all")
        v_all = persist.tile([128, NT, HL, 65], BF, name="v_all")  # [t, j, h, d|1]
        att_sb = persist.tile([128, 2, S], BF, name="att_sb")  # [d-pair, dg, s]
        keep_sb = persist.tile([128, nm, kw], BF, name="keep_sb")
        qb_sb = persist.tile([128, 2], F32, name="qb_sb")
        kb_sb = persist.tile([128, 2], F32, name="kb_sb")
        wq_sb = persist.tile([128, NEP, 2, 256], F8, name="wq_sb")
        wk_sb = persist.tile([128, NEP, 2, 256], F8, name="wk_sb")
        wv_sb = persist.tile([128, NEP, 2, 256], F8, name="wv_sb")
        g_bc = persist.tile([128, E], F32, name="g_bc")
        b_bc = persist.tile([128, E], F32, name="b_bc")
        magic = persist.tile([128, 1], mybir.dt.uint32, name="magic")
        wo_sb = persist.tile([128, 2, E], F8, name="wo_sb")    # [d-pair, dg, e]
        resid_sb = persist.tile([128, NS, E], F16, name="resid_sb")

        nc.sync.dma_start(out=wq_sb, in_=wq_v)
        nc.sync.dma_start(out=wk_sb, in_=wk_v)
        nc.sync.dma_start(out=wv_sb, in_=wv_v)
        nc.sync.dma_start(out=qb_sb, in_=qb_v)
        nc.sync.dma_start(out=kb_sb, in_=kb_v)
        nc.gpsimd.dma_start(out=g_bc, in_=lng_v[0:1, :].to_broadcast([128, E]))
        nc.gpsimd.dma_start(out=b_bc, in_=lnb_v[0:1, :].to_broadcast([128, E]))
        nc.vector.memset(magic, 0x5F3759DF)
        # ones column for the row-sum trick
        nc.vector.memset(v_all[:, :, :, 64:65], 1.0)

        # ---- phase 1: QKV projections (fp8 DoubleRow) -------------------
        with tc.tile_pool(name="xT", bufs=3 * NEP) as xpool, \
             tc.tile_pool(name="p1", bufs=3, space="PSUM") as p1, \
             tc.tile_pool(name="p1v", bufs=3, space="PSUM") as p1v:
            def load_chunks(x_v_):
                chunks = []
                for ep in range(NEP):
                    xc = xpool.tile([128, 2, S], F8, name="xc", tag="xc")
                    nc.sync.dma_start(out=xc, in_=x_v_[:, ep, :, :])
                    chunks.append(xc)
                return chunks

            def qk_proj(chunks, w_sb_, bias_, scale_, dst_, sb):
                if True:
                    for dg in range(2):
                        ps = p1.tile([128, SBLK], F32, name="ps", tag="p1")
                        if USE_DR:
                            for s2 in range(2):
                                for ep in range(NEP):
                                    cs = slice(s2 * 256, (s2 + 1) * 256)
                                    nc.tensor.matmul(
                                        ps[:, cs],
                                        w_sb_[:, ep, :, dg * 128:(dg + 1) * 128],
                                        chunks[ep][:, :,
                                                   sb * SBLK + s2 * 256:
                                                   sb * SBLK + s2 * 256 + 256],
                                        start=(ep == 0), stop=(ep == NEP - 1),
                                        perf_mode=mybir.MatmulPerfMode.DoubleRow,
                                    )
                        else:
                            for ep in range(NEP):
                                for j in range(2):
                                    nc.tensor.matmul(
                                        ps,
                                        w_sb_[:, ep, j, dg * 128:(dg + 1) * 128],
                                        chunks[ep][:, j,
                                                   sb * SBLK:(sb + 1) * SBLK],
                                        start=(ep == 0 and j == 0),
                                        stop=(ep == NEP - 1 and j == 1),
                                    )
                        nc.scalar.activation(
                            dst_[:, dg, sb * SBLK:(sb + 1) * SBLK], ps,
                            Act.Identity, bias=bias_[:, dg:dg + 1], scale=scale_,
                        )

            nc.sync.dma_start(out=keep_sb, in_=keep_v)
            nc.sync.dma_start(out=wo_sb, in_=wo_v)
            qchunks = load_chunks(qx_v)
            kchunks = load_chunks(kx_v)
            vchunks = load_chunks(vx_v)
            nc.sync.dma_start(out=resid_sb, in_=res_v)

            def v_proj(jlist):
                # V (form A): v[t, d] = value[t, :] @ WvT
                for j in jlist:
                    ps = p1v.tile([128, 256], F32, name="psv", tag="p1v")
                    if USE_DR:
                        for ep in range(NEP):
                            nc.tensor.matmul(
                                ps,
                                vchunks[ep][:, :, j * TBLK:(j + 1) * TBLK],
                                wv_sb[:, ep, :, :],
                                start=(ep == 0), stop=(ep == NEP - 1),
                                perf_mode=mybir.MatmulPerfMode.DoubleRow,
                            )
                    else:
                        for ep in range(NEP):
                            for jj in range(2):
                                nc.tensor.matmul(
                                    ps,
                                    vchunks[ep][:, jj, j * TBLK:(j + 1) * TBLK],
                                    wv_sb[:, ep, jj, :],
                                    start=(ep == 0 and jj == 0),
                                    stop=(ep == NEP - 1 and jj == 1),
                                )
                    nc.scalar.activation(
                        v_all[:, j, :, 0:64],
                        ps.rearrange("p (h d) -> p h d", h=HL),
                        Act.Identity, scale=1.0 / W_SCALE,
                    )

            def k_proj(sb):
                for dg in range(2):
                    ps = p1.tile([128, SBLK], F32, name="ps", tag="p1")
                    if USE_DR:
                        for s2 in range(2):
                            for ep in range(NEP):
                                cs = slice(s2 * 256, (s2 + 1) * 256)
                                nc.tensor.matmul(
                                    ps[:, cs],
                                    wk_sb[:, ep, :, dg * 128:(dg + 1) * 128],
                                    kchunks[ep][:, :,
                                                sb * SBLK + s2 * 256:
                                                sb * SBLK + s2 * 256 + 256],
                                    start=(ep == 0), stop=(ep == NEP - 1),
                                    perf_mode=mybir.MatmulPerfMode.DoubleRow,
                                )
                    else:
                        for ep in range(NEP):
                            for j in range(2):
                                nc.tensor.matmul(
                                    ps,
                                    wk_sb[:, ep, j, dg * 128:(dg + 1) * 128],
                                    kchunks[ep][:, j, sb * SBLK:(sb + 1) * SBLK],
                                    start=(ep == 0 and j == 0),
                                    stop=(ep == NEP - 1 and j == 1),
                                )
                    nc.scalar.activation(
                        k_all[:, dg, sb * SBLK:(sb + 1) * SBLK], ps,
                        Act.Identity, bias=kb_sb[:, dg:dg + 1],
                        scale=1.0 / W_SCALE,
                    )

            for sb_ in range(NS):
                qk_proj(qchunks, wq_sb, qb_sb, 0.125 / W_SCALE, q_all, sb_)
            # emit in the order attention(sb) consumes: k(sb) + v(4sb..4sb+3)
            for sb_ in range(NS):
                k_proj(sb_)
                v_proj(range(4 * sb_, 4 * sb_ + 4))

        # ---- phase 2: attention; proj + RS + LN pipelined per sblock ----
        sc_pool = ctx.enter_context(tc.tile_pool(name="sc", bufs=2, space="PSUM"))
        ctx_pool = ctx.enter_context(tc.tile_pool(name="ctxp", bufs=3, space="PSUM"))
        pr_pool = ctx.enter_context(tc.tile_pool(name="prp", bufs=1, space="PSUM"))
        probs_pool = ctx.enter_context(tc.tile_pool(name="probs", bufs=6))
        small = ctx.enter_context(tc.tile_pool(name="small", bufs=4))
        work = ctx.enter_context(tc.tile_pool(name="work", bufs=4))

        def attention(sb):
            tlist = tiles[sb]
            for dg in range(2):
                cA = ctx_pool.tile([65, SBLK], F32, name="cA", tag="ctx")
                cB = ctx_pool.tile([65, SBLK], F32, name="cB", tag="ctx")
                for idx, (j, c_lo, band) in enumerate(tlist):
                    first, last = idx == 0, idx == len(tlist) - 1
                    # two-bank scores tile: head A in half 0, head B in half 1
                    sc2 = sc_pool.tile([128, 2, SBLK], F32, name="sc2", tag="sc")
                    ts = slice(j * TBLK, (j + 1) * TBLK)
                    ss = slice(sb * SBLK + c_lo, (sb + 1) * SBLK)
                    cs = slice(c_lo, SBLK)
                    nc.tensor.matmul(sc2[:, 0, cs], k_all[0:64, dg, ts],
                                     q_all[0:64, dg, ss], start=True, stop=True)
                    nc.tensor.matmul(sc2[:, 1, cs], k_all[64:128, dg, ts],
                                     q_all[64:128, dg, ss], start=True, stop=True)
                    p2 = probs_pool.tile([128, 2, SBLK], BF, name="p2", tag="pr")
                    nc.scalar.activation(p2[:, :, cs], sc2[:, :, cs], Act.Exp)
                    if band is not None:
                        b0, b1, mi = band
                        bs_ = slice(b0, b1)
                        keep_b = keep_sb[:, mi:mi + 1, 0:b1 - b0].to_broadcast(
                            [128, 2, b1 - b0])
                        nc.vector.tensor_mul(p2[:, :, bs_], p2[:, :, bs_], keep_b)
                    nc.tensor.matmul(cA[:, cs], v_all[:, j, 2 * dg, :], p2[:, 0, cs],
                                     start=first, stop=last)
                    nc.tensor.matmul(cB[:, cs], v_all[:, j, 2 * dg + 1, :],
                                     p2[:, 1, cs], start=first, stop=last)
                for i, cx in ((0, cA), (1, cB)):
                    sums = small.tile([1, SBLK], F32, name="sums", tag="sums")
                    nc.vector.tensor_copy(sums, cx[64:65, :])
                    recip = small.tile([1, SBLK], F32, name="recip", tag="recip")
                    nc.vector.reciprocal_approx_fast(recip, sums)
                    bc = work.tile([64, SBLK], F32, name="bc", tag="bc")
                    nc.gpsimd.partition_broadcast(bc, recip, channels=64)
                    nc.vector.tensor_mul(
                        att_sb[64 * i:64 * i + 64, dg, sb * SBLK:(sb + 1) * SBLK],
                        cx[0:64, :], bc)

        GROUPS_OF = {"rs4": [[0], [1], [2], [3]], "rs2": [[0, 1], [2, 3]],
                     "rs1": [[0, 1, 2, 3]], "rs31": [[0, 1, 2], [3]],
                     "none": [[0], [1], [2], [3]]}[collective]
        sb_group = {sb: grp for grp in GROUPS_OF for sb in grp}

        def proj_rs(sb):
            # out projection partials + ReduceScatter for this sblock.
            # For the last sblock attention is finished: use the freed scores
            # banks (2-bank tiles) and the idle ACT engine to shorten the tail.
            last = sb == NS - 1
            for sc in range(4):   # 128-row chunks within sblock
                srow = sb * 4 + sc
                row = slice(srow * 128, (srow + 1) * 128)
                # dest slot so that rank r of the (possibly merged) RS
                # receives rows [sb0-own, sb1-own, ...] in sb-major order,
                # matching rs_out's [sb*128:(sb+1)*128] layout.
                grp = sb_group[sb]
                dst = grp[0] * 512 + sc * (len(grp) * 128) + \
                    grp.index(sb) * 128
                pcopy = work.tile([128, E], BF, name="pcopy", tag="pcopy")
                if last:
                    pp2 = sc_pool.tile([128, 2, SBLK], F32, name="pp2", tag="sc")
                    for eo in range(2):
                        for dg in range(2):
                            nc.tensor.matmul(
                                pp2[:, eo, :],
                                att_sb[:, dg, row],
                                wo_sb[:, dg, eo * SBLK:(eo + 1) * SBLK],
                                start=(dg == 0), stop=(dg == 1),
                            )
                    nc.scalar.activation(
                        pcopy.rearrange("p (c s) -> p c s", c=2), pp2,
                        Act.Identity, scale=1.0 / W_SCALE)
                else:
                    for eo in range(2):
                        pp = pr_pool.tile([128, SBLK], F32, name="pp", tag="pp")
                        for dg in range(2):
                            nc.tensor.matmul(
                                pp,
                                att_sb[:, dg, row],
                                wo_sb[:, dg, eo * SBLK:(eo + 1) * SBLK],
                                start=(dg == 0), stop=(dg == 1),
                            )
                        nc.vector.tensor_scalar_mul(
                            pcopy[:, eo * SBLK:(eo + 1) * SBLK], pp,
                            1.0 / W_SCALE)
                nc.sync.dma_start(out=rs_in[dst:dst + 128, :], in_=pcopy)
            if collective == "none":
                nc.sync.dma_start(out=rs_out[sb * 128:(sb + 1) * 128, :],
                                  in_=rs_in[sb * SBLK:sb * SBLK + 128, :])
            elif sb == sb_group[sb][-1]:
                grp = sb_group[sb]
                lo, n = grp[0], len(grp)
                nc.gpsimd.collective_compute(
                    "ReduceScatter", Alu.add,
                    ins=[rs_in[lo * SBLK:(lo + n) * SBLK, :]],
                    outs=[rs_out[lo * 128:(lo + n) * 128, :]],
                    replica_groups=GROUPS,
                )

        def post_ln(sb):
            # residual + LN on own 128 rows of this sblock (runs one sblock
            # behind the RS so its waits never head-of-line block the queues)
            pchunk = work.tile([128, E], BF, name="pchunk", tag="pchunk")
            x_t = work.tile([128, E], F32, name="x_t", tag="x_t")
            stats = small.tile([128, 2, 6], F32, name="stats", tag="stats")
            for h in range(2):
                hs = slice(h * 512, (h + 1) * 512)
                nc.sync.dma_start(out=pchunk[:, hs],
                                  in_=rs_out[sb * 128:(sb + 1) * 128, hs])
                nc.vector.tensor_add(x_t[:, hs], resid_sb[:, sb, hs], pchunk[:, hs])
                nc.vector.bn_stats(stats[:, h, :], x_t[:, hs])
            mv = small.tile([128, 2], F32, name="mv", tag="mv")
            nc.vector.bn_aggr(mv, stats)
            # rstd = rsqrt(var + eps) on DVE (bit-trick seed + 2 Newton iters)
            # so ACT never leaves the exp table set
            U32 = mybir.dt.uint32
            ws = small.tile([128, 1], F32, name="ws", tag="ws")
            nc.vector.tensor_scalar_add(ws, mv[:, 1:2], LN_EPS)
            hbits = small.tile([128, 1], U32, name="hbits", tag="hbits")
            nc.vector.tensor_scalar(hbits, ws.bitcast(U32), 1, None,
                                    op0=Alu.logical_shift_right)
            rstd = small.tile([128, 1], F32, name="rstd", tag="rstd")
            nc.vector.scalar_tensor_tensor(
                rstd.bitcast(U32), magic, 0, hbits, op0=Alu.bypass,
                op1=Alu.subtract)
            nt = small.tile([128, 1], F32, name="nt", tag="nt")
            for _ in range(2):
                nc.vector.tensor_mul(nt, ws, rstd)
                nc.vector.tensor_mul(nt, nt, rstd)
                nc.vector.tensor_scalar(nt, nt, -0.5, 1.5, op0=Alu.mult, op1=Alu.add)
                nc.vector.tensor_mul(rstd, rstd, nt)
            y_t = work.tile([128, E], F32, name="y_t", tag="y_t")
            nc.vector.scalar_tensor_tensor(
                y_t, x_t, mv[:, 0:1], g_bc, op0=Alu.subtract, op1=Alu.mult)
            o_t = work.tile([128, E], F16, name="o_t", tag="o_t")
            nc.vector.scalar_tensor_tensor(
                o_t, y_t, rstd, b_bc, op0=Alu.mult, op1=Alu.add)
            nc.sync.dma_start(out=out_v[sb * 128:(sb + 1) * 128, :], in_=o_t)

        # Interleaved schedule: project this sblock's q/k/v, then its
        # attention (exp on ACT overlaps the next sblock's projection PE
        # work), then the PREVIOUS sblock's out-projection + RS (so the DVE
        # normalize of this sblock overlaps PE), with LN lagging the RS.
        emitted_ln = 0
        for sb in range(NS):
            attention(sb)
            proj_rs(sb)
            # LN lags one sblock behind RS readiness so its waits never
            # head-of-line block the queues
            if collective == "none":
                ready = sb + 1
            else:
                ready = 0
                for grp_ in GROUPS_OF:
                    if grp_[-1] <= sb:
                        ready = grp_[-1] + 1
            while emitted_ln < min(ready, sb):
                post_ln(emitted_ln)
                emitted_ln += 1
        for sb2 in range(emitted_ln, NS):
            post_ln(sb2)

    nc.finalize()
    return nc


def _prep_core(inputs, b, g, mult_list, kw):
    heads = slice(HL * g, HL * (g + 1))
    mask = np.asarray(inputs["mask"], bool)
    query = np.asarray(inputs["query"][b], np.float32)
    key = np.asarray(inputs["key"][b], np.float32)
    value = np.asarray(inputs["value"][b], np.float32)
    Wq_w = np.asarray(inputs["Wq_w"], np.float32)
    Wk_w = np.asarray(inputs["Wk_w"], np.float32)
    Wv_w = np.asarray(inputs["Wv_w"], np.float32)
    Wq_b = np.asarray(inputs["Wq_b"], np.float32)
    Wk_b = np.asarray(inputs["Wk_b"], np.float32)
    out_w = np.asarray(inputs["out_w"], np.float32)
    out_b = np.asarray(inputs["out_b"], np.float32)
    Wv_b = np.asarray(inputs["Wv_b"], np.float32)
    nm = max(1, len(mult_list))

    scale = np.float32(1.0 / np.sqrt(Dh))

    def packb(t):  # [4, 64] -> [128, 2] pair-major
        return np.ascontiguousarray(
            t.reshape(2, 2, Dh).transpose(1, 2, 0).reshape(128, 2))

    def wdev(W):  # [E, 256] -> [128, NEP, 2, 256] (e = ep*256 + j*128 + p)
        return np.ascontiguousarray(
            (W * W_SCALE).reshape(NEP, 2, 128, 256).transpose(2, 0, 1, 3))

    blob8 = np.empty(N8, F8NP)
    blob8[O8_Q:O8_Q + SZ_X] = np.ascontiguousarray(query.T).astype(F8NP).ravel()
    blob8[O8_K:O8_K + SZ_X] = np.ascontiguousarray(key.T).astype(F8NP).ravel()
    blob8[O8_V:O8_V + SZ_X] = np.ascontiguousarray(value.T).astype(F8NP).ravel()
    blob8[O8_WQ:O8_WQ + SZ_W] = \
        wdev(Wq_w[heads].reshape(256, E).T).astype(F8NP).ravel()
    blob8[O8_WK:O8_WK + SZ_W] = \
        wdev(Wk_w[heads].reshape(256, E).T).astype(F8NP).ravel()
    blob8[O8_WV:O8_WV + SZ_W] = \
        wdev(Wv_w[heads].reshape(256, E).T).astype(F8NP).ravel()
    # wo: [128, 2, E] with partition p = att-dim within dg half, x W_SCALE
    wo = out_w[:, 256 * g:256 * (g + 1)].T * W_SCALE   # [256, E]
    blob8[O8_WO:O8_WO + SZ_W] = np.ascontiguousarray(
        wo.reshape(2, 128, E).transpose(1, 0, 2)).astype(F8NP).ravel()

    # keep: [128, nm, kw] bf16, viewed as f16 bytes in blob16
    keep = np.zeros((nm, TBLK, kw), np.float32)
    for mi, (sb, j, b0, b1) in enumerate(mult_list):
        reg = mask[b, sb * SBLK + b0:sb * SBLK + b1, j * TBLK:(j + 1) * TBLK]
        keep[mi, :, 0:b1 - b0] = (~reg).T.astype(np.float32)
    keep_dev = np.ascontiguousarray(keep.transpose(1, 0, 2)).astype(BF16)

    const = out_b + Wv_b.reshape(E) @ out_w.T
    rows = query.reshape(NS, 4, 128, E)[:, g, :, :].reshape(512, E)
    resid = (rows + const[None, :]).reshape(NS, 128, E).transpose(1, 0, 2)

    blob16 = np.empty(SZ_RES + nm * 128 * kw, np.float16)
    blob16[0:SZ_RES] = np.ascontiguousarray(resid).astype(np.float16).ravel()
    blob16[SZ_RES:] = keep_dev.ravel().view(np.float16)

    blob32 = np.empty(2560, np.float32)
    blob32[0:256] = packb(Wq_b[heads] * scale).ravel()
    blob32[256:512] = packb(Wk_b[heads]).ravel()
    blob32[512:1536] = np.asarray(inputs["ln_g"], np.float32)
    blob32[1536:2560] = np.asarray(inputs["ln_b"], np.float32)

    return {"blob8": blob8, "blob16": blob16, "blob32": blob32}


def prep_in_maps(inputs):
    mask = np.asarray(inputs["mask"], bool)
    tiles, mult_list, kw = classify_mask(mask)
    return [_prep_core(inputs, c // 4, c % 4, mult_list, kw)
            for c in range(N_CORES)]


COLLECTIVE = "rs4"


def kernel(**inputs):
    mask = np.asarray(inputs["mask"], bool)
    tiles, mult_list, kw = classify_mask(mask)
    key_struct = (tiles, mult_list, kw, COLLECTIVE, USE_DR)
    if key_struct not in _BUILD_CACHE:
        _BUILD_CACHE[key_struct] = build(tiles, mult_list, kw,
                                         collective=COLLECTIVE)
    nc = _BUILD_CACHE[key_struct]

    in_maps = prep_in_maps(inputs)
    res = run_bass_kernel_spmd(nc, in_maps, core_ids=list(range(N_CORES)))

    out = np.empty((B, S, E), np.float32)
    for c in range(N_CORES):
        b, g = c // 4, c % 4
        o = np.asarray(res.results[c]["out"]).reshape(512, E).astype(np.float32)
        for sb in range(NS):
            out[b, sb * SBLK + 128 * g: sb * SBLK + 128 * (g + 1), :] = \
                o[sb * 128:(sb + 1) * 128, :]
    return out
